# revision 1
# baseline (speedup 1.0000x reference)
"""Trainium2 Bass kernel v2 for nn_GammaNeuronNet — restructured pipeline.

Changes vs baseline:
* V-only exchange: per step only the core's own V' slice (512 values) is
  AllGathered. The s-state update is elementwise, so every core computes the
  FULL s/sE vectors redundantly in the exchange layout -- no s exchange, no
  transposes at all.
* den/num constants (G_leak+co_gap, G_leak*E_leak) are folded into the matmul
  accumulation via an extra "ones" k-tile (hi/lo bf16 split for accuracy).
* Elementwise V update runs on [1,512] DVE ops reading PSUM directly.
* MM burst ordered syn-tiles-first / gap-tiles-last so the AllGather of V
  overlaps the next step's syn half.
* Optional column tiling (NGRP=2/4): concurrent matmul streams into separate
  PSUM partition groups, combined with cross-quadrant DVE adds.
* Optional keep-warm dummy matmuls to stop HAM clock-gate oscillation.

Layouts:
* "xw" layout: [128, 32] tile, neuron n=32p+t at partition p, col t. The
  8-core AllGather of per-core [1,512] slices lands exactly in this layout.
* Own-slice layout: [1,512], col j = own neuron 512c+j; PSUM matvec output
  columns are in this order.
"""

import os
import numpy as np
import ml_dtypes

N = 4096
NCORES = 8
ROWS = N // NCORES            # 512 matrix rows per core
KTM = N // 128                # 32 k-tiles per matrix
KT = 2 * KTM                  # 64 merged k-tiles (G_syn then G_gap)
BETA, V_TH, A_R, A_D = 0.125, -15.0, 1.0, 5.0

NGRP = int(os.environ.get("GAMMA_NGRP", "2"))       # column-tiling groups
NDUMMY = int(os.environ.get("GAMMA_NDUMMY", "38"))   # keep-warm dummy MMs/step

_cache = {}
last_results = None


def _n_steps(timestep, runtime):
    t, n = 0.0, 0
    while t < runtime:
        t += timestep
        n += 1
    return n


def _build(n_steps: int, dt: float, fast: bool, ndummy: int):
    import concourse.bacc as bacc
    import concourse.mybir as mybir
    import concourse.tile as tile

    f32 = mybir.dt.float32
    bf16 = mybir.dt.bfloat16

    nc = bacc.Bacc("TRN2", target_bir_lowering=False, debug=False,
                   num_devices=NCORES)

    w_d = nc.dram_tensor("w_in", [128, KT * ROWS], bf16, kind="ExternalInput")
    s2_d = nc.dram_tensor("s2_0", [128, 64], bf16, kind="ExternalInput")
    vg_d = nc.dram_tensor("vg0", [128, 32], bf16, kind="ExternalInput")
    v0_d = nc.dram_tensor("v0", [1, 512], f32, kind="ExternalInput")
    s0_d = nc.dram_tensor("s0", [128, 32], f32, kind="ExternalInput")
    esyn_d = nc.dram_tensor("esyn", [128, 32], f32, kind="ExternalInput")
    cst_d = nc.dram_tensor("cst", [128, 512], bf16, kind="ExternalInput")
    idl_d = nc.dram_tensor("idl", [128, 2], bf16, kind="ExternalInput")
    vout_d = nc.dram_tensor("v_out", [1, 512], f32, kind="ExternalOutput")

    rg = [list(range(NCORES))]
    Sigmoid = mybir.ActivationFunctionType.Sigmoid
    Copy = mybir.ActivationFunctionType.Copy

    ar_dt = float(A_R) * dt
    c1 = 1.0 - float(A_D) * dt
    sig_scale = float(BETA)
    sig_bias = -float(BETA) * float(V_TH)
    inv_dt = 1.0 / dt


    with tile.TileContext(nc) as tc:
        with (
            tc.tile_pool(name="const", bufs=1) as constp,
            tc.tile_pool(name="wpool", bufs=1) as wp,
            tc.tile_pool(name="state", bufs=2) as stp,
            tc.tile_pool(name="ew", bufs=2) as ewp,
            tc.tile_pool(name="mm", bufs=2, space="PSUM") as mmp,
            tc.tile_pool(name="tot", bufs=2, space="PSUM") as totp,
            tc.tile_pool(name="dum", bufs=1, space="PSUM") as dump,
            tc.tile_pool(name="dram", bufs=2, space="DRAM") as dramp,
        ):
            w_sb = wp.tile([128, KT * ROWS], bf16)
            nc.sync.dma_start(w_sb[:], w_d[:])
            esyn_sb = constp.tile([128, 32], f32)
            nc.sync.dma_start(esyn_sb[:], esyn_d[:])
            cst_sb = constp.tile([128, 512], bf16)
            nc.sync.dma_start(cst_sb[:], cst_d[:])
            idl_sb = constp.tile([128, 2], bf16)
            nc.sync.dma_start(idl_sb[:], idl_d[:])
            sigb_sb = constp.tile([128, 1], f32)
            nc.vector.memset(sigb_sb[:], sig_bias)

            # state double buffers
            s2b = [stp.tile([128, 64], bf16, tag="s2", name=f"s2b{j}")
                   for j in range(2)]
            gvb = [stp.tile([128, 32], bf16, tag="gv", name=f"gvb{j}")
                   for j in range(2)]
            sfb = [stp.tile([128, 32], f32, tag="sf", name=f"sfb{j}")
                   for j in range(2)]
            nc.sync.dma_start(s2b[0][:], s2_d[:])
            nc.sync.dma_start(gvb[0][:], vg_d[:])
            nc.sync.dma_start(sfb[0][:], s0_d[:])

            # all [1,512]-row scratch lives in [128,*] tiles so every AP is
            # guaranteed to start at partition 0 (verifier requirement)
            vb = stp.tile([128, 1024], f32, name="vb")       # vown parity halves
            ewr = stp.tile([128, 2560], f32, name="ewr")     # p,m,r,q,dv rows
            vxr = stp.tile([128, 512], bf16, name="vxr")     # bf16 exchange row
            numr = stp.tile([128, 512], f32, name="numr")    # ACT-extracted num
            cpb = stp.tile([128, 1536], f32, name="cpb")     # combine scratch
            nc.sync.dma_start(vb[0:1, 0:512], v0_d[:])

            ccin_bufs = [dramp.tile([1, 512], bf16, tag="ccin",
                                    name=f"ccinb{j}") for j in range(2)]

            for i in range(n_steps):
                last = i == n_steps - 1
                s2 = s2b[i % 2]
                gv = gvb[i % 2]

                mm = mmp.tile([128, 512], f32, tag="mm")

                # ---- matvec burst, M=1 column-tiled groups:
                #   g0 (psum@0):  den = G_syn@s, even k-tiles + c0 constants
                #   g1 (psum@32): den partial, odd k-tiles
                #   g2 (psum@64): num = G_syn@sE + G_gap@V, even + gle consts
                #   g3 (psum@96): num partial, odd k-tiles
                # syn tiles first; gap tiles (need V from the AllGather) last.
                nc.tensor.matmul(mm[0:1, :], idl_sb[:, 0:1], cst_sb[:, :],
                                 start=True, stop=False, tile_position=(0, 0))
                nc.tensor.matmul(mm[64:65, :], idl_sb[:, 1:2], cst_sb[:, :],
                                 start=True, stop=False, tile_position=(0, 64))
                for s in range(16):
                    ke, ko = 2 * s, 2 * s + 1
                    nc.tensor.matmul(
                        mm[0:1, :], s2[:, ke:ke + 1],
                        w_sb[:, ke * ROWS:(ke + 1) * ROWS],
                        start=False, stop=(s == 15), tile_position=(0, 0))
                    nc.tensor.matmul(
                        mm[32:33, :], s2[:, ko:ko + 1],
                        w_sb[:, ko * ROWS:(ko + 1) * ROWS],
                        start=(s == 0), stop=(s == 15), tile_position=(0, 32))
                    nc.tensor.matmul(
                        mm[64:65, :], s2[:, 32 + ke:33 + ke],
                        w_sb[:, ke * ROWS:(ke + 1) * ROWS],
                        start=False, stop=False, tile_position=(0, 64))
                    nc.tensor.matmul(
                        mm[96:97, :], s2[:, 32 + ko:33 + ko],
                        w_sb[:, ko * ROWS:(ko + 1) * ROWS],
                        start=(s == 0), stop=False, tile_position=(0, 96))
                # den partials complete here -> den-side EW can start early
                cpd = cpb[0:1, 0:512]
                den = cpb[0:1, 512:1024]
                nc.vector.tensor_copy(cpd, mm[32:33, :])
                nc.vector.tensor_add(den, mm[0:1, :], cpd)
                for s in range(16):
                    ke, ko = 32 + 2 * s, 32 + 2 * s + 1
                    nc.tensor.matmul(
                        mm[64:65, :], gv[:, ke - 32:ke - 31],
                        w_sb[:, ke * ROWS:(ke + 1) * ROWS],
                        start=False, stop=(s == 15), tile_position=(0, 64))
                    nc.tensor.matmul(
                        mm[96:97, :], gv[:, ko - 32:ko - 31],
                        w_sb[:, ko * ROWS:(ko + 1) * ROWS],
                        start=False, stop=(s == 15), tile_position=(0, 96))
                cpn = cpb[0:1, 1024:1536]
                numt = numr[0:1, 0:512]
                nc.scalar.activation(cpn, mm[96:97, :], Copy,
                                     bias=0.0, scale=1.0)

                # ---- V update on [1,512] rows (all partition 0):
                #      vstep = (num - V*den) * min(dt, 1/den)
                vown = vb[0:1, 512 * (i % 2):512 * (i % 2) + 512]
                vnew = vb[0:1, 512 * ((i + 1) % 2):512 * ((i + 1) % 2) + 512]
                p = ewr[0:1, 0:512]
                m = ewr[0:1, 512:1024]
                r = ewr[0:1, 1024:1536]
                q = ewr[0:1, 1536:2048]
                dv = ewr[0:1, 2048:2560]
                vx = vxr[0:1, :]
                num = numt
                if fast:
                    # weights prescaled by dt on host; den*dt<1 guaranteed so
                    # the clip never binds: V' = (V - V*den') + num'.
                    # p/w1 depend only on den -> run during the gap MMs;
                    # after the last gap MM only t=add(mm64,w1) (DVE) and
                    # cpn (ACT, parallel) then vx remain on the cycle.
                    w1 = ewr[0:1, 512:1024]
                    t_ = ewr[0:1, 1024:1536]
                    nc.vector.tensor_mul(p, vown, den)
                    nc.vector.tensor_sub(w1, vown, p)
                    nc.vector.tensor_add(t_, mm[64:65, :], w1)
                    if last:
                        nc.vector.tensor_add(vnew, t_, cpn)
                        nc.sync.dma_start(vout_d[:], vnew)
                        break
                    nc.vector.tensor_add(vx, t_, cpn)
                    nc.vector.tensor_add(vnew, t_, cpn)
                else:
                    nc.vector.tensor_mul(p, vown, den)
                    nc.vector.tensor_scalar_max(m, den, inv_dt)
                    nc.vector.reciprocal(r, m)
                    nc.vector.tensor_sub(q, num, p)
                    nc.vector.tensor_mul(dv, q, r)
                if not fast:
                    if last:
                        nc.vector.tensor_add(vnew, vown, dv)
                        nc.sync.dma_start(vout_d[:], vnew)
                        break
                    nc.vector.tensor_add(vx, vown, dv)
                    nc.vector.tensor_add(vnew, vown, dv)

                # ---- exchange own V' (bf16, 1KB) -> full V in xw layout
                ccin = ccin_bufs[i % 2]
                nc.sync.dma_start(ccin[:], vx)
                ccout = nc.dram_tensor(f"ccout{i}", [128, 32], bf16,
                                       addr_space="Shared")
                nc.gpsimd.collective_compute(
                    "AllGather", mybir.AluOpType.bypass, replica_groups=rg,
                    ins=[ccin[:].opt()], outs=[ccout[:].opt()])
                nc.sync.dma_start(gvb[(i + 1) % 2][:], ccout[:])

                # ---- s chain for step i+1 (redundant full-N, hidden under
                #      the MM burst / AG window)
                sig = ewp.tile([128, 32], f32, tag="sig")
                u = ewp.tile([128, 32], f32, tag="u")
                w_ = ewp.tile([128, 32], f32, tag="w")
                p2 = ewp.tile([128, 32], f32, tag="p2")
                snew = sfb[(i + 1) % 2]
                s2n = s2b[(i + 1) % 2]
                nc.scalar.activation(sig[:], gvb[i % 2][:], Sigmoid,
                                     bias=sigb_sb[:, 0:1], scale=sig_scale)
                nc.scalar.activation(u[:], sig[:], Copy, bias=0.0,
                                     scale=ar_dt)
                nc.scalar.activation(w_[:], u[:], Copy, bias=c1, scale=-1.0)
                nc.vector.tensor_mul(p2[:], sfb[i % 2][:], w_[:])
                nc.vector.tensor_add(snew[:], p2[:], u[:])
                nc.vector.tensor_copy(s2n[:, 0:32], snew[:])
                nc.vector.tensor_mul(s2n[:, 32:64], snew[:], esyn_sb[:])

                # ---- keep-warm dummy matmuls (fill the AG wait window)
                if ndummy:
                    dps = dump.tile([2, 512], f32, tag="dummy")
                    for k in range(ndummy):
                        nc.tensor.matmul(dps[0:2, :], idl_sb[:, 0:2],
                                         cst_sb[:, :], start=True, stop=True,
                                         tile_position=(0, 0),
                                         skip_group_check=True)

    nc.compile()
    return nc


def _prep(input_V, G_leak, E_leak, G_syn, E_syn, G_gap, dt, fast):
    iv = np.asarray(input_V, np.float32).reshape(-1)
    G_leak = np.asarray(G_leak, np.float32)
    E_leak = np.asarray(E_leak, np.float32)
    G_syn = np.asarray(G_syn, np.float32)
    E_syn = np.asarray(E_syn, np.float32)
    G_gap = np.asarray(G_gap, np.float32)
    in_len = iv.shape[0]

    in_avg = np.float32(iv.mean(dtype=np.float32))
    V0 = np.concatenate([iv, np.full(N - in_len, in_avg, np.float32)])
    x = (BETA * (V0 - V_TH)).astype(np.float32)
    sig = (1.0 / (1.0 + np.exp(-x, dtype=np.float32))).astype(np.float32)
    s0 = (A_R * sig / (A_R * sig + A_D)).astype(np.float32)
    sE0 = (s0 * E_syn).astype(np.float32)
    co_gap = G_gap.sum(axis=1, dtype=np.float32)
    c0_full = (G_leak + co_gap).astype(np.float32)
    gle_full = (G_leak * E_leak).astype(np.float32)

    wscale = np.float32(dt) if fast else np.float32(1.0)
    Gs16 = (G_syn * wscale).astype(ml_dtypes.bfloat16)
    Gg16 = (G_gap * wscale).astype(ml_dtypes.bfloat16)
    c0_full = c0_full * wscale
    gle_full = gle_full * wscale

    def hilo(v):
        hi = v.astype(ml_dtypes.bfloat16)
        lo = (v - hi.astype(np.float32)).astype(ml_dtypes.bfloat16)
        return hi, lo

    def xw(v):
        # full-N vector -> [128, 32] xw layout (neuron 32p+t at (p, t))
        return np.ascontiguousarray(v.reshape(128, 32))

    idl = np.zeros((128, 2), ml_dtypes.bfloat16)
    idl[0, 0] = idl[1, 0] = 1.0
    idl[2, 1] = idl[3, 1] = 1.0

    s2_0 = np.zeros((128, 64), ml_dtypes.bfloat16)
    s2_0[:, 0:32] = xw(s0)
    s2_0[:, 32:64] = xw(sE0)

    in_maps = []
    for c in range(NCORES):
        rows = slice(c * ROWS, (c + 1) * ROWS)
        A_s = Gs16[rows, :].reshape(ROWS, 128, 32)   # [n, p, t], k = 32p + t
        A_g = Gg16[rows, :].reshape(ROWS, 128, 32)
        Ws = np.transpose(A_s, (1, 2, 0))            # [p, t, n]
        Wg = np.transpose(A_g, (1, 2, 0))
        W = np.ascontiguousarray(
            np.concatenate([Ws, Wg], axis=1)
        ).reshape(128, KT * ROWS)

        c0hi, c0lo = hilo(c0_full[rows])
        glehi, glelo = hilo(gle_full[rows])
        cst = np.zeros((128, 512), ml_dtypes.bfloat16)
        cst[0] = c0hi
        cst[1] = c0lo
        cst[2] = glehi
        cst[3] = glelo

        in_maps.append({
            "w_in": W,
            "s2_0": s2_0,
            "vg0": xw(V0).astype(ml_dtypes.bfloat16),
            "v0": np.ascontiguousarray(V0[rows]).reshape(1, 512),
            "s0": xw(s0),
            "esyn": xw(E_syn),
            "cst": cst,
            "idl": idl,
        })
    return in_maps, in_len


def kernel(input_V, G_leak, E_leak, G_syn, E_syn, G_gap, timestep, runtime):
    global last_results
    from concourse.bass_utils import run_bass_kernel_spmd

    dt = float(np.asarray(timestep))
    rt = float(np.asarray(runtime))
    n_steps = _n_steps(dt, rt)

    # den*dt stays below 1 iff leak+gap+syn conductances are small enough;
    # then clip(dV*dt, +-|V_inf-V|) == dV*dt exactly and the kernel can skip
    # the reciprocal/min entirely (weights prescaled by dt instead).
    G_leak_a = np.asarray(G_leak, np.float32)
    G_syn_a = np.asarray(G_syn, np.float32)
    G_gap_a = np.asarray(G_gap, np.float32)
    V0x = np.asarray(input_V, np.float32)
    s_bound = max(0.21, float(A_R / (A_R + A_D)) + 0.05)
    den_bound = float((G_leak_a + G_gap_a.sum(1) +
                       G_syn_a.sum(1) * s_bound).max()) * dt
    fast = den_bound < 0.95

    key = (n_steps, dt, fast, NDUMMY)
    if key not in _cache:
        _cache[key] = _build(n_steps, dt, fast, NDUMMY)
    nc = _cache[key]

    in_maps, in_len = _prep(input_V, G_leak, E_leak, G_syn, E_syn, G_gap,
                            dt, fast)
    trace = os.environ.get("GAMMA_TRACE", "0") == "1"
    res = run_bass_kernel_spmd(
        nc, in_maps, core_ids=list(range(NCORES)), trace=trace
    )
    last_results = res

    V = np.concatenate(
        [np.asarray(res.results[c]["v_out"]).reshape(ROWS)
         for c in range(NCORES)]
    ).astype(np.float32)
    V[in_len:] = 0.0
    return V

